# revision 8
# baseline (speedup 1.0000x reference)
"""AttentionSubsample Trainium2 kernel — data-parallel over batch on 8 cores.

Reference computation (per batch b of 512):
  kv = BN(x @ w_kv.T)            x:[196,256] -> kv:[196,384] -> k:[8,196,16], v:[8,196,32]
  q  = BN(xs @ w_q.T)            xs = spatial 2x subsample (49 tokens) -> q:[8,49,16]
  attn = softmax(q @ k.T * scale + ab)     ab:[8,49,196]
  out = BN(hardswish(attn @ v) @ w_p.T)    -> [49,512]

Device strategy (per core, 64 batches):
  - host folds BN into weights/biases, pre-transposes weights, pre-gathers ab,
    folds softmax scale into w_q and hardswish /6 into w_p
  - x shipped pre-transposed (feature-major) in bf16: xT [2,128,196]
  - kT feature-major via matmul; qT feature-major (padded to 32-aligned heads);
    v token-major via matmul with xT as stationary
  - scores token-major [49,196] per head; ab preloaded into PSUM via identity
    matmul; exp (no max subtraction; verified safe range) with accum_out = Z
  - attn and hs transposed via DMA-transpose (bf16 xbar)
  - attn@v and final projection as plain matmuls; epilogue fused on DVE/ACT
"""

import numpy as np
import ml_dtypes

import concourse.bass as bass
import concourse.tile as tile
from concourse import bacc, mybir
from concourse.bass_utils import run_bass_kernel_spmd

BF16 = mybir.dt.bfloat16
F32 = mybir.dt.float32

B, N, NQ, C = 512, 196, 49, 256
H = 8
NCORES = 8
BPC = B // NCORES
EPS = 1e-5
SCALE = 16 ** -0.5
AF = mybir.ActivationFunctionType
ALU = mybir.AluOpType

bf16 = ml_dtypes.bfloat16


def build_core(nbatch=BPC):
    nc = bacc.Bacc("TRN2", target_bir_lowering=False, debug=False)

    xt_d = nc.dram_tensor("xt", [nbatch, 2, 128, N], BF16, kind="ExternalInput")
    wkt_d = nc.dram_tensor("wkt", [2, 128, 128], BF16, kind="ExternalInput")
    wqt_d = nc.dram_tensor("wqt", [2, 128, 128], BF16, kind="ExternalInput")
    wvt_d = nc.dram_tensor("wvt", [2, 128, 256], BF16, kind="ExternalInput")
    wpt_d = nc.dram_tensor("wpt", [2, 128, 512], BF16, kind="ExternalInput")
    tk_d = nc.dram_tensor("tk", [128, 1], F32, kind="ExternalInput")
    tq_d = nc.dram_tensor("tq", [128, 1], F32, kind="ExternalInput")
    qac_d = nc.dram_tensor("qac", [49, 8, 49], BF16, kind="ExternalInput")
    abs_d = nc.dram_tensor("abs", [49, 8, 196], BF16, kind="ExternalInput")
    tvb_d = nc.dram_tensor("tvb", [49, 256], F32, kind="ExternalInput")
    tpb_d = nc.dram_tensor("tpb", [49, 512], F32, kind="ExternalInput")
    out_d = nc.dram_tensor("out", [nbatch, 49, 512], F32, kind="ExternalOutput")

    with tile.TileContext(nc) as tc:
        with (
            tc.tile_pool(name="consts", bufs=1) as consts,
            tc.tile_pool(name="io", bufs=3) as io,
            tc.tile_pool(name="work", bufs=3) as work,
            tc.tile_pool(name="attnp", bufs=2) as attnp,
            tc.tile_pool(name="stackp", bufs=1) as stackp,
            tc.tile_pool(name="ps_kq", bufs=2, space="PSUM") as ps_kq,
            tc.tile_pool(name="ps_v", bufs=2, space="PSUM") as ps_v,
            tc.tile_pool(name="ps_s", bufs=2, space="PSUM") as ps_s,
            tc.tile_pool(name="ps_o", bufs=1, space="PSUM") as ps_o,
            tc.tile_pool(name="ps_p", bufs=1, space="PSUM") as ps_p,
        ):
            wkt_sb = consts.tile([128, 2, 128], BF16)
            wqt_sb = consts.tile([128, 2, 128], BF16)
            wvt_sb = consts.tile([128, 2, 256], BF16)
            wpt_sb = consts.tile([128, 2, 512], BF16)
            for c in range(2):
                nc.scalar.dma_start(out=wkt_sb[:, c, :], in_=wkt_d[c])
                nc.scalar.dma_start(out=wqt_sb[:, c, :], in_=wqt_d[c])
                nc.scalar.dma_start(out=wvt_sb[:, c, :], in_=wvt_d[c])
                nc.scalar.dma_start(out=wpt_sb[:, c, :], in_=wpt_d[c])
            tk_sb = consts.tile([128, 1], F32)
            tq_sb = consts.tile([128, 1], F32)
            nc.scalar.dma_start(out=tk_sb, in_=tk_d[:])
            nc.scalar.dma_start(out=tq_sb, in_=tq_d[:])
            qa_sb = stackp.tile([65, 8, 49], BF16)
            ka_sb = stackp.tile([65, 8, 196], BF16)
            nc.scalar.dma_start(out=qa_sb[16:65, :, :], in_=qac_d[:])
            nc.scalar.dma_start(out=ka_sb[16:65, :, :], in_=abs_d[:])
            tvb_sb = consts.tile([49, 256], F32)
            nc.scalar.dma_start(out=tvb_sb, in_=tvb_d[:])
            tpb_sb = consts.tile([49, 512], F32)
            nc.scalar.dma_start(out=tpb_sb, in_=tpb_d[:])
            three_sb = consts.tile([49, 1], F32)
            nc.vector.memset(three_sb, 3.0)

            for b in range(nbatch):
                # ---- load xT (feature-major x) ----
                xt_sb = io.tile([128, 2, N], BF16)
                for c in range(2):
                    nc.scalar.dma_start(out=xt_sb[:, c, :], in_=xt_d[b, c])

                # subsampled tokens view: [128, 7, 7] (stride-2 rows/cols)
                def xs_view(c):
                    v = xt_sb[:, c, :].rearrange(
                        "p (a s b2 t) -> p a s b2 t", a=7, s=2, b2=7, t=2
                    )
                    return v[:, :, 0, :, 0]

                # ---- kT [128,196] and qT (padded) [128,2,49] ----
                kq_ps = ps_kq.tile([128, 294], F32)
                for c in range(2):
                    nc.tensor.matmul(
                        kq_ps[:, 0:196], lhsT=wkt_sb[:, c, :], rhs=xt_sb[:, c, :],
                        start=(c == 0), stop=(c == 1),
                    )
                for c in range(2):
                    nc.tensor.matmul(
                        kq_ps[:, 196:245],
                        lhsT=wqt_sb[:, c, :],
                        rhs=xs_view(c),
                        start=(c == 0), stop=(c == 1),
                    )
                kT_sb = work.tile([128, N], BF16)
                nc.vector.tensor_scalar_add(kT_sb, kq_ps[:, 0:196], tk_sb)
                qT_sb = work.tile([128, 49], BF16)
                nc.vector.tensor_scalar_add(qT_sb, kq_ps[:, 196:245], tq_sb)
                for h in range(H):
                    nc.scalar.dma_start(
                        out=qa_sb[0:16, h, :], in_=qT_sb[16 * h:16 * h + 16, :])
                    nc.scalar.dma_start(
                        out=ka_sb[0:16, h, :], in_=kT_sb[16 * h:16 * h + 16, :])

                # ---- v token-major [196,256] (two chunks in one PSUM bank) ----
                v_ps = ps_v.tile([128, 512], F32)
                for c in range(2):
                    nc.tensor.matmul(
                        v_ps[:, 0:256], lhsT=xt_sb[:, c, 0:128], rhs=wvt_sb[:, c, :],
                        start=(c == 0), stop=(c == 1),
                    )
                for c in range(2):
                    nc.tensor.matmul(
                        v_ps[0:68, 256:512], lhsT=xt_sb[:, c, 128:196],
                        rhs=wvt_sb[:, c, :],
                        start=(c == 0), stop=(c == 1),
                    )
                v0_sb = work.tile([128, 256], BF16)
                v1_sb = work.tile([68, 256], BF16)
                nc.vector.tensor_copy(v0_sb, v_ps[:, 0:256])
                nc.vector.tensor_copy(v1_sb, v_ps[0:68, 256:512])

                # ---- scores + softmax (no max; range verified) ----
                attn_sb = attnp.tile([64, 8, 256], BF16)
                z_sb = work.tile([49, 8], F32)
                for j in range(4):
                    s_ps = ps_s.tile([49, 392], F32)
                    for jj in range(2):
                        h = 2 * j + jj
                        nc.tensor.matmul(
                            s_ps[:, 196 * jj:196 * jj + 196],
                            lhsT=qa_sb[:, h, :],
                            rhs=ka_sb[:, h, :],
                            start=True, stop=True,
                        )
                    for jj in range(2):
                        h = 2 * j + jj
                        nc.scalar.activation(
                            out=attn_sb[0:49, h, 0:196],
                            in_=s_ps[:, 196 * jj:196 * jj + 196],
                            func=AF.Exp,
                            accum_out=z_sb[:, h:h + 1],
                        )
                zr_sb = work.tile([49, 8], F32)
                nc.vector.reciprocal(zr_sb, z_sb)

                # ---- transpose attn (DMA xbar, bf16) ----
                aT_sb = attnp.tile([128, 8, 2, 64], BF16)
                for h in range(H):
                    for cc in range(2):
                        nc.sync.dma_start(
                            out=aT_sb[:, h, cc, :],
                            in_=attn_sb[:, h, 128 * cc:128 * cc + 128],
                            transpose=True,
                        )

                # ---- attn @ v -> [49, 256] ----
                o_ps = ps_o.tile([49, 256], F32)
                for h in range(H):
                    nc.tensor.matmul(
                        o_ps[:, 32 * h:32 * h + 32],
                        lhsT=aT_sb[:, h, 0, 0:49], rhs=v0_sb[:, 32 * h:32 * h + 32],
                        start=True, stop=False,
                    )
                    nc.tensor.matmul(
                        o_ps[:, 32 * h:32 * h + 32],
                        lhsT=aT_sb[0:68, h, 1, 0:49], rhs=v1_sb[:, 32 * h:32 * h + 32],
                        start=False, stop=True,
                    )

                # ---- normalize, +bias_v, hardswish ----
                y_sb = work.tile([49, 256], F32)
                for h in range(H):
                    nc.vector.scalar_tensor_tensor(
                        out=y_sb[:, 32 * h:32 * h + 32],
                        in0=o_ps[:, 32 * h:32 * h + 32],
                        scalar=zr_sb[:, h:h + 1],
                        in1=tvb_sb[:, 32 * h:32 * h + 32],
                        op0=ALU.mult, op1=ALU.add,
                    )
                r_sb = work.tile([49, 256], F32)
                nc.scalar.activation(r_sb, y_sb, AF.Relu, bias=three_sb, scale=1.0)
                hs_sb = work.tile([64, 256], BF16)
                nc.vector.scalar_tensor_tensor(
                    out=hs_sb[0:49, :], in0=r_sb, scalar=6.0, in1=y_sb,
                    op0=ALU.min, op1=ALU.mult,
                )

                # ---- transpose hs, final projection ----
                hsT_sb = work.tile([128, 2, 64], BF16)
                for cc in range(2):
                    nc.sync.dma_start(
                        out=hsT_sb[:, cc, :],
                        in_=hs_sb[:, 128 * cc:128 * cc + 128],

                        transpose=True,
                    )
                p_ps = ps_p.tile([49, 512], F32)
                for cc in range(2):
                    nc.tensor.matmul(
                        p_ps, lhsT=hsT_sb[:, cc, 0:49], rhs=wpt_sb[:, cc, :],
                        start=(cc == 0), stop=(cc == 1),
                    )
                out_sb = io.tile([49, 512], F32)
                nc.vector.tensor_add(out_sb, p_ps, tpb_sb)
                nc.scalar.dma_start(out=out_d[b], in_=out_sb)

    nc.compile()
    return nc


def _build_bias_idxs():
    import itertools
    points = list(itertools.product(range(14), range(14)))
    points_ = list(itertools.product(range(7), range(7)))
    offsets, idxs = {}, []
    for p1 in points_:
        for p2 in points:
            off = (abs(p1[0] * 2 - p2[0]), abs(p1[1] * 2 - p2[1]))
            if off not in offsets:
                offsets[off] = len(offsets)
            idxs.append(offsets[off])
    return np.array(idxs, dtype=np.int32).reshape(NQ, N)


def make_inputs(x, w_kv, kv_g, kv_b, kv_m, kv_v, w_q, q_g, q_b, q_m, q_v,
                w_p, p_g, p_b, p_m, p_v, ab_table, bias_idxs, nbatch=BPC,
                ncores=NCORES):
    """Host-side preprocessing -> list of per-core input dicts."""
    f = np.float32
    x = np.asarray(x, f)
    s_kv = np.asarray(kv_g, f) / np.sqrt(np.asarray(kv_v, f) + EPS)
    wkv = np.asarray(w_kv, f) * s_kv[:, None]
    tkv = np.asarray(kv_b, f) - np.asarray(kv_m, f) * s_kv
    wkv_h = wkv.reshape(H, 48, C)
    tkv_h = tkv.reshape(H, 48)
    w_k = wkv_h[:, :16, :].reshape(128, C)
    t_k = tkv_h[:, :16].reshape(128)
    w_v = wkv_h[:, 16:, :].reshape(256, C)
    t_v = tkv_h[:, 16:].reshape(256)

    s_q = np.asarray(q_g, f) / np.sqrt(np.asarray(q_v, f) + EPS)
    wq = np.asarray(w_q, f) * (s_q * SCALE)[:, None]
    t_q = (np.asarray(q_b, f) - np.asarray(q_m, f) * s_q) * SCALE

    s_p = np.asarray(p_g, f) / np.sqrt(np.asarray(p_v, f) + EPS)
    wp = np.asarray(w_p, f) * s_p[:, None] / 6.0
    t_p = np.asarray(p_b, f) - np.asarray(p_m, f) * s_p

    idxs = _build_bias_idxs()
    ab = np.asarray(ab_table, f)[:, idxs]                       # [8,49,196]
    ab_s = np.ascontiguousarray(ab.transpose(1, 0, 2))          # [49,8,196]
    qa_c = np.ascontiguousarray(
        np.broadcast_to(np.eye(NQ, dtype=f)[:, None, :], (NQ, H, NQ)))

    base = dict(
        wkt=np.ascontiguousarray(w_k.T.reshape(2, 128, 128)).astype(bf16),
        wqt=np.ascontiguousarray(wq.T.reshape(2, 128, 128)).astype(bf16),
        wvt=np.ascontiguousarray(w_v.T.reshape(2, 128, 256)).astype(bf16),
        wpt=np.ascontiguousarray(wp.T.reshape(2, 128, 512)).astype(bf16),
        tk=np.ascontiguousarray(t_k[:, None]),
        tq=np.ascontiguousarray(t_q[:, None]),
        qac=qa_c.astype(bf16),
        abs=ab_s.astype(bf16),
        tvb=np.ascontiguousarray(np.broadcast_to(t_v, (NQ, 256))),
        tpb=np.ascontiguousarray(np.broadcast_to(t_p, (NQ, 512))),
    )

    xt = x.transpose(0, 2, 1).astype(bf16).reshape(B, 2, 128, N)
    in_maps = []
    for cid in range(ncores):
        m = dict(base)
        m["xt"] = np.ascontiguousarray(xt[cid * nbatch:(cid + 1) * nbatch])
        in_maps.append(m)
    return in_maps


_NC_CACHE = {}
LAST_RESULT = None


def kernel(**inputs):
    if "nc" not in _NC_CACHE:
        _NC_CACHE["nc"] = build_core(BPC)
    nc = _NC_CACHE["nc"]
    in_maps = make_inputs(**inputs)
    res = run_bass_kernel_spmd(nc, in_maps, core_ids=list(range(NCORES)))
    global LAST_RESULT
    LAST_RESULT = res
    out = np.concatenate([r["out"] for r in res.results], axis=0)
    return out.astype(np.float32)


# revision 10
# speedup vs baseline: 4.8150x; 4.8150x over previous
"""AttentionSubsample Trainium2 kernel — data-parallel over batch on 8 cores.

Reference computation (per batch b of 512):
  kv = BN(x @ w_kv.T)            x:[196,256] -> kv:[196,384] -> k:[8,196,16], v:[8,196,32]
  q  = BN(xs @ w_q.T)            xs = spatial 2x subsample (49 tokens) -> q:[8,49,16]
  attn = softmax(q @ k.T * scale + ab)     ab:[8,49,196]
  out = BN(hardswish(attn @ v) @ w_p.T)    -> [49,512]

Device strategy (per core, 64 batches):
  - host folds BN into weights/biases, pre-transposes weights, pre-gathers ab,
    folds softmax scale into w_q and hardswish /6 into w_p
  - x shipped pre-transposed (feature-major) in bf16: xT [2,128,196]
  - kT feature-major via matmul; qT feature-major (padded to 32-aligned heads);
    v token-major via matmul with xT as stationary
  - scores token-major [49,196] per head; ab preloaded into PSUM via identity
    matmul; exp (no max subtraction; verified safe range) with accum_out = Z
  - attn and hs transposed via DMA-transpose (bf16 xbar)
  - attn@v and final projection as plain matmuls; epilogue fused on DVE/ACT
"""

import numpy as np
import ml_dtypes

import concourse.bass as bass
import concourse.tile as tile
from concourse import bacc, mybir
from concourse.bass_utils import run_bass_kernel_spmd

BF16 = mybir.dt.bfloat16
F32 = mybir.dt.float32

B, N, NQ, C = 512, 196, 49, 256
H = 8
NCORES = 8
BPC = B // NCORES
EPS = 1e-5
SCALE = 16 ** -0.5
AF = mybir.ActivationFunctionType
ALU = mybir.AluOpType

bf16 = ml_dtypes.bfloat16


def build_core(nbatch=BPC):
    nc = bacc.Bacc("TRN2", target_bir_lowering=False, debug=False)

    xt_d = nc.dram_tensor("xt", [nbatch, 2, 128, N], BF16, kind="ExternalInput")
    wkt_d = nc.dram_tensor("wkt", [2, 128, 128], BF16, kind="ExternalInput")
    wqt_d = nc.dram_tensor("wqt", [2, 128, 128], BF16, kind="ExternalInput")
    wvt_d = nc.dram_tensor("wvt", [2, 128, 256], BF16, kind="ExternalInput")
    wpt_d = nc.dram_tensor("wpt", [2, 128, 512], BF16, kind="ExternalInput")
    tk_d = nc.dram_tensor("tk", [128, 1], F32, kind="ExternalInput")
    tq_d = nc.dram_tensor("tq", [128, 1], F32, kind="ExternalInput")
    stk_d = nc.dram_tensor("stk", [49, 8, 245], BF16, kind="ExternalInput")
    id64_d = nc.dram_tensor("id64", [64, 64], BF16, kind="ExternalInput")
    tvb_d = nc.dram_tensor("tvb", [49, 256], F32, kind="ExternalInput")
    tpb_d = nc.dram_tensor("tpb", [49, 512], F32, kind="ExternalInput")
    out_d = nc.dram_tensor("out", [nbatch, 49, 512], F32, kind="ExternalOutput")

    with tile.TileContext(nc) as tc:
        with (
            tc.tile_pool(name="consts", bufs=1) as consts,
            tc.tile_pool(name="io", bufs=3) as io,
            tc.tile_pool(name="work", bufs=3) as work,
            tc.tile_pool(name="attnp", bufs=2) as attnp,
            tc.tile_pool(name="stackp", bufs=1) as stackp,
            tc.tile_pool(name="ps_kq", bufs=1, space="PSUM") as ps_kq,
            tc.tile_pool(name="ps_v", bufs=1, space="PSUM") as ps_v,
            tc.tile_pool(name="ps_s", bufs=2, space="PSUM") as ps_s,
            tc.tile_pool(name="ps_t", bufs=2, space="PSUM") as ps_t,
            tc.tile_pool(name="ps_o", bufs=1, space="PSUM") as ps_o,
            tc.tile_pool(name="ps_p", bufs=1, space="PSUM") as ps_p,
        ):
            wkt_sb = consts.tile([128, 2, 128], BF16)
            wqt_sb = consts.tile([128, 2, 128], BF16)
            wvt_sb = consts.tile([128, 2, 256], BF16)
            wpt_sb = consts.tile([128, 2, 512], BF16)
            for c in range(2):
                nc.scalar.dma_start(out=wkt_sb[:, c, :], in_=wkt_d[c])
                nc.scalar.dma_start(out=wqt_sb[:, c, :], in_=wqt_d[c])
                nc.scalar.dma_start(out=wvt_sb[:, c, :], in_=wvt_d[c])
                nc.scalar.dma_start(out=wpt_sb[:, c, :], in_=wpt_d[c])
            tk_sb = consts.tile([128, 1], F32)
            tq_sb = consts.tile([128, 1], F32)
            nc.scalar.dma_start(out=tk_sb, in_=tk_d[:])
            nc.scalar.dma_start(out=tq_sb, in_=tq_d[:])
            qka_sb = stackp.tile([65, 8, 245], BF16)
            nc.scalar.dma_start(out=qka_sb[16:65, :, :], in_=stk_d[:])
            id64_sb = consts.tile([64, 64], BF16)
            nc.scalar.dma_start(out=id64_sb, in_=id64_d[:])
            tvb_sb = consts.tile([49, 256], F32)
            nc.scalar.dma_start(out=tvb_sb, in_=tvb_d[:])
            tpb_sb = consts.tile([49, 512], F32)
            nc.scalar.dma_start(out=tpb_sb, in_=tpb_d[:])
            three_sb = consts.tile([49, 1], F32)
            nc.vector.memset(three_sb, 3.0)

            for b in range(nbatch):
                # ---- load xT (feature-major x) ----
                xt_sb = io.tile([128, 2, N], BF16)
                for c in range(2):
                    nc.scalar.dma_start(out=xt_sb[:, c, :], in_=xt_d[b, c])

                # subsampled tokens view: [128, 7, 7] (stride-2 rows/cols)
                def xs_view(c):
                    v = xt_sb[:, c, :].rearrange(
                        "p (a s b2 t) -> p a s b2 t", a=7, s=2, b2=7, t=2
                    )
                    return v[:, :, 0, :, 0]

                # ---- kT [128,196] and qT (padded) [128,2,49] ----
                kq_ps = ps_kq.tile([128, 294], F32)
                for c in range(2):
                    nc.tensor.matmul(
                        kq_ps[:, 0:196], lhsT=wkt_sb[:, c, :], rhs=xt_sb[:, c, :],
                        start=(c == 0), stop=(c == 1),
                    )
                for c in range(2):
                    nc.tensor.matmul(
                        kq_ps[:, 196:245],
                        lhsT=wqt_sb[:, c, :],
                        rhs=xs_view(c),
                        start=(c == 0), stop=(c == 1),
                    )
                qkT_sb = work.tile([128, 245], BF16)
                nc.vector.tensor_scalar_add(
                    qkT_sb[:, 0:196], kq_ps[:, 0:196], tk_sb)
                nc.vector.tensor_scalar_add(
                    qkT_sb[:, 196:245], kq_ps[:, 196:245], tq_sb)
                for h in range(H):
                    nc.sync.dma_start(
                        out=qka_sb[0:16, h, :], in_=qkT_sb[16 * h:16 * h + 16, :])

                # ---- v token-major [196,256] (two chunks in one PSUM bank) ----
                v_ps = ps_v.tile([128, 512], F32)
                for c in range(2):
                    nc.tensor.matmul(
                        v_ps[:, 0:256], lhsT=xt_sb[:, c, 0:128], rhs=wvt_sb[:, c, :],
                        start=(c == 0), stop=(c == 1),
                    )
                for c in range(2):
                    nc.tensor.matmul(
                        v_ps[0:68, 256:512], lhsT=xt_sb[:, c, 128:196],
                        rhs=wvt_sb[:, c, :],
                        start=(c == 0), stop=(c == 1),
                    )
                v0_sb = work.tile([128, 256], BF16)
                v1_sb = work.tile([68, 256], BF16)
                nc.vector.tensor_copy(v0_sb, v_ps[:, 0:256])
                nc.vector.tensor_copy(v1_sb, v_ps[0:68, 256:512])

                # ---- scores + softmax (no max; range verified) ----
                attn_sb = attnp.tile([64, 8, 256], BF16)
                z_sb = work.tile([49, 8], F32)
                for j in range(4):
                    s_ps = ps_s.tile([49, 392], F32)
                    for jj in range(2):
                        h = 2 * j + jj
                        nc.tensor.matmul(
                            s_ps[:, 196 * jj:196 * jj + 196],
                            lhsT=qka_sb[:, h, 196:245],
                            rhs=qka_sb[:, h, 0:196],
                            start=True, stop=True,
                        )
                    for jj in range(2):
                        h = 2 * j + jj
                        nc.scalar.activation(
                            out=attn_sb[0:49, h, 0:196],
                            in_=s_ps[:, 196 * jj:196 * jj + 196],
                            func=AF.Exp,
                            accum_out=z_sb[:, h:h + 1],
                        )
                zr_sb = work.tile([49, 8], F32)
                nc.vector.reciprocal(zr_sb, z_sb)

                # ---- transpose attn on PE via identity ----
                taT = ps_t.tile([128, 16, 64], BF16, tag="t")
                for h in range(H):
                    nc.tensor.transpose(
                        taT[:, h, :], attn_sb[:, h, 0:128], id64_sb)
                    nc.tensor.transpose(
                        taT[:, 8 + h, :], attn_sb[:, h, 128:256], id64_sb)
                aT_sb = attnp.tile([128, 8, 2, 49], BF16)
                nc.vector.tensor_copy(aT_sb[:, :, 0, :], taT[:, 0:8, 0:49])
                nc.scalar.activation(aT_sb[:, :, 1, :], taT[:, 8:16, 0:49], AF.Copy)

                # ---- attn @ v -> [49, 256] ----
                o_ps = ps_o.tile([49, 256], F32)
                for h in range(H):
                    nc.tensor.matmul(
                        o_ps[:, 32 * h:32 * h + 32],
                        lhsT=aT_sb[:, h, 0, :], rhs=v0_sb[:, 32 * h:32 * h + 32],
                        start=True, stop=False,
                    )
                    nc.tensor.matmul(
                        o_ps[:, 32 * h:32 * h + 32],
                        lhsT=aT_sb[0:68, h, 1, :], rhs=v1_sb[:, 32 * h:32 * h + 32],
                        start=False, stop=True,
                    )

                # ---- normalize, +bias_v, hardswish ----
                y_sb = work.tile([49, 256], F32)
                for h in range(H):
                    nc.vector.scalar_tensor_tensor(
                        out=y_sb[:, 32 * h:32 * h + 32],
                        in0=o_ps[:, 32 * h:32 * h + 32],
                        scalar=zr_sb[:, h:h + 1],
                        in1=tvb_sb[:, 32 * h:32 * h + 32],
                        op0=ALU.mult, op1=ALU.add,
                    )
                r_sb = work.tile([49, 256], F32)
                nc.scalar.activation(r_sb, y_sb, AF.Relu, bias=three_sb, scale=1.0)
                hs_sb = work.tile([64, 256], BF16)
                nc.vector.scalar_tensor_tensor(
                    out=hs_sb[0:49, :], in0=r_sb, scalar=6.0, in1=y_sb,
                    op0=ALU.min, op1=ALU.mult,
                )

                # ---- transpose hs on PE, final projection ----
                thsT = ps_t.tile([128, 16, 64], BF16, tag="t")
                for cc in range(2):
                    nc.tensor.transpose(
                        thsT[:, cc, :], hs_sb[:, 128 * cc:128 * cc + 128],
                        id64_sb)
                hsT_sb = work.tile([128, 2, 49], BF16)
                nc.vector.tensor_copy(hsT_sb, thsT[:, 0:2, 0:49])
                p_ps = ps_p.tile([49, 512], F32)
                for cc in range(2):
                    nc.tensor.matmul(
                        p_ps, lhsT=hsT_sb[:, cc, :], rhs=wpt_sb[:, cc, :],
                        start=(cc == 0), stop=(cc == 1),
                    )
                out_sb = io.tile([49, 512], F32)
                nc.vector.tensor_add(out_sb, p_ps, tpb_sb)
                nc.scalar.dma_start(out=out_d[b], in_=out_sb)

    nc.compile()
    return nc


def _build_bias_idxs():
    import itertools
    points = list(itertools.product(range(14), range(14)))
    points_ = list(itertools.product(range(7), range(7)))
    offsets, idxs = {}, []
    for p1 in points_:
        for p2 in points:
            off = (abs(p1[0] * 2 - p2[0]), abs(p1[1] * 2 - p2[1]))
            if off not in offsets:
                offsets[off] = len(offsets)
            idxs.append(offsets[off])
    return np.array(idxs, dtype=np.int32).reshape(NQ, N)


def make_inputs(x, w_kv, kv_g, kv_b, kv_m, kv_v, w_q, q_g, q_b, q_m, q_v,
                w_p, p_g, p_b, p_m, p_v, ab_table, bias_idxs, nbatch=BPC,
                ncores=NCORES):
    """Host-side preprocessing -> list of per-core input dicts."""
    f = np.float32
    x = np.asarray(x, f)
    s_kv = np.asarray(kv_g, f) / np.sqrt(np.asarray(kv_v, f) + EPS)
    wkv = np.asarray(w_kv, f) * s_kv[:, None]
    tkv = np.asarray(kv_b, f) - np.asarray(kv_m, f) * s_kv
    wkv_h = wkv.reshape(H, 48, C)
    tkv_h = tkv.reshape(H, 48)
    w_k = wkv_h[:, :16, :].reshape(128, C)
    t_k = tkv_h[:, :16].reshape(128)
    w_v = wkv_h[:, 16:, :].reshape(256, C)
    t_v = tkv_h[:, 16:].reshape(256)

    s_q = np.asarray(q_g, f) / np.sqrt(np.asarray(q_v, f) + EPS)
    wq = np.asarray(w_q, f) * (s_q * SCALE)[:, None]
    t_q = (np.asarray(q_b, f) - np.asarray(q_m, f) * s_q) * SCALE

    s_p = np.asarray(p_g, f) / np.sqrt(np.asarray(p_v, f) + EPS)
    wp = np.asarray(w_p, f) * s_p[:, None] / 6.0
    t_p = np.asarray(p_b, f) - np.asarray(p_m, f) * s_p

    idxs = _build_bias_idxs()
    ab = np.asarray(ab_table, f)[:, idxs]                       # [8,49,196]
    ab_s = ab.transpose(1, 0, 2)                                # [49,8,196]
    qa_c = np.broadcast_to(np.eye(NQ, dtype=f)[:, None, :], (NQ, H, NQ))
    stk = np.ascontiguousarray(np.concatenate([ab_s, qa_c], axis=2))

    base = dict(
        wkt=np.ascontiguousarray(w_k.T.reshape(2, 128, 128)).astype(bf16),
        wqt=np.ascontiguousarray(wq.T.reshape(2, 128, 128)).astype(bf16),
        wvt=np.ascontiguousarray(w_v.T.reshape(2, 128, 256)).astype(bf16),
        wpt=np.ascontiguousarray(wp.T.reshape(2, 128, 512)).astype(bf16),
        tk=np.ascontiguousarray(t_k[:, None]),
        tq=np.ascontiguousarray(t_q[:, None]),
        stk=stk.astype(bf16),
        id64=np.eye(64, dtype=f).astype(bf16),
        tvb=np.ascontiguousarray(np.broadcast_to(t_v, (NQ, 256))),
        tpb=np.ascontiguousarray(np.broadcast_to(t_p, (NQ, 512))),
    )

    xt = x.transpose(0, 2, 1).astype(bf16).reshape(B, 2, 128, N)
    in_maps = []
    for cid in range(ncores):
        m = dict(base)
        m["xt"] = np.ascontiguousarray(xt[cid * nbatch:(cid + 1) * nbatch])
        in_maps.append(m)
    return in_maps


_NC_CACHE = {}
LAST_RESULT = None


def kernel(**inputs):
    if "nc" not in _NC_CACHE:
        _NC_CACHE["nc"] = build_core(BPC)
    nc = _NC_CACHE["nc"]
    in_maps = make_inputs(**inputs)
    res = run_bass_kernel_spmd(nc, in_maps, core_ids=list(range(NCORES)))
    global LAST_RESULT
    LAST_RESULT = res
    out = np.concatenate([r["out"] for r in res.results], axis=0)
    return out.astype(np.float32)
